# revision 17
# baseline (speedup 1.0000x reference)
"""Trainium2 Bass kernel for the delta-rule memory recurrence (DeltaNet-style).

Full-input contract: kernel(memory, key, value) -> final memory, all np.ndarray,
shapes (16,256,256), (16,4096,256), (16,4096,256) -> (16,256,256) float32.

Strategy: pure data-parallel over batch (2 batches per NeuronCore x 8 cores).
Per batch the sequential recurrence

    kn   = k_t / ||k_t||
    M   <- M - (1.1 * M kn - 0.1 * v_t) kn^T

is reformulated chunkwise (C=128 steps per chunk) via the WY / UT transform.
With L = 1.1 * strict_lower(Kn Kn^T) the chunk solve is

    T  = (I + L)^{-1} ~= (I - L)(I + L^2)(I + L^4 + L^8)     [exact thru L^11]
    W  = 0.1 * T^T Kn                    (C x DK, state-independent)
    Y  = Kn Mt                           (C x DV)
    R  = V - 11 * Y
    Mt <- Mt + W^T R                     (Mt = M^T state, (DK, DV))

All precompute (Gram, powers, T, W) is state-independent; the kernel is a
per-chunk modulo software pipeline so every iteration interleaves one chunk
of the sequential state chain with staged precompute of later chunks,
keeping the PE stream dense. PSUM accumulation tricks: L^8 accumulates onto
the L^4 bank (Q = I+L4+L8 with a diag fill), T accumulates onto X2's bank
((I-L)X2 - I with a diag fill), avoiding identity matmuls and extra drains.
All HBM inputs are pre-cast to fp16 on host; DMA loads are batched 4 chunks
per descriptor.
"""

import numpy as np

import concourse.bass as bass
import concourse.mybir as mybir
import concourse.tile as tile
from concourse.bass import ts
from concourse.bass_utils import run_bass_kernel_spmd
from concourse.masks import make_identity

F32 = mybir.dt.float32
F16 = mybir.dt.float16
AOP = mybir.AluOpType

B, S, DK, DV = 16, 4096, 256, 256
NCORES = 8
BLOC = B // NCORES          # batches per core
C = 128                     # chunk length
LR = 0.1
AC = 1.0 + LR               # 1.1


def _split_waits(nc, max_waits=1):
    """walrus codegen on this toolchain encodes at most one semaphore wait per
    instruction; hoist excess waits onto same-engine NoOps placed just before."""
    n_split = 0
    for f in nc.m.functions:
        for bb in f.blocks:
            insts = bb.instructions
            out = []
            for inst in insts:
                si = getattr(inst, "sync_info", None)
                w = list(si.on_wait) if (si and si.on_wait) else []
                k = 0
                while len(w) > max_waits:
                    head, w = w[:max_waits], w[max_waits:]
                    out.append(mybir.InstNoOp(
                        name=f"{inst.name}-wsplit{k}",
                        engine=inst.engine,
                        sync_info=mybir.SyncInfo(on_wait=head, on_update=[]),
                    ))
                    n_split += 1
                    k += 1
                if k:
                    inst.sync_info = mybir.SyncInfo(
                        on_wait=w, on_update=list(si.on_update or [])
                    )
                out.append(inst)
            bb.instructions = out
    return n_split


def build_nc(s_loc=S, split=True):
    nch = s_loc // C
    nbk = nch // 4              # 4-chunk DMA blocks
    nc = bass.Bass()
    memT = nc.declare_dram_parameter("memT", [BLOC, DK, DV], F32, isOutput=False)
    key_d = nc.declare_dram_parameter("key", [BLOC, s_loc, DK], F16, isOutput=False)
    keyT_d = nc.declare_dram_parameter("keyT", [BLOC, DK, s_loc], F16,
                                       isOutput=False)
    val_d = nc.declare_dram_parameter("value", [BLOC, s_loc, DV], F16,
                                      isOutput=False)
    outT = nc.declare_dram_parameter("outT", [BLOC, DK, DV], F32, isOutput=True)

    from contextlib import ExitStack
    with tile.TileContext(nc) as tc:
        with ExitStack() as stack:
            ep = lambda *a, **kw: stack.enter_context(tc.tile_pool(*a, **kw))
            consts = ep(name="consts", bufs=1)
            kv = ep(name="kv", bufs=3)
            vv = ep(name="vv", bufs=3)
            ktp = ep(name="kt", bufs=3)
            anp = ep(name="an", bufs=2)
            mskp = ep(name="msk", bufs=12)
            p2p = ep(name="p2s", bufs=6)
            p4p = ep(name="p4s", bufs=3)
            qp = ep(name="qs", bufs=3)
            x2p = ep(name="x2s", bufs=2)
            ttp = ep(name="tts", bufs=2)
            wp = ep(name="ws", bufs=4)
            statep = ep(name="state", bufs=4)
            mtp = ep(name="mt", bufs=5)
            mtinitp = ep(name="mtinit", bufs=1)
            # PSUM: 8 banks total.  mt 2 + y 1 + gpw 2 + p4q 2 + xt 1
            ps_gpw = ep(name="ps_gpw", bufs=2, space="PSUM")
            ps_mid = ep(name="ps_mid", bufs=2, space="PSUM")
            ps_y = ep(name="ps_y", bufs=2, space="PSUM")
            ps_mt0 = ep(name="ps_mt0", bufs=1, space="PSUM")
            ps_mt1 = ep(name="ps_mt1", bufs=1, space="PSUM")
            one_reg = nc.gpsimd.to_reg(1.0)
            ident32 = consts.tile([128, 128], F32, tag="ident32")
            make_identity(nc, ident32)
            ident16 = consts.tile([128, 128], F16, tag="ident16")
            make_identity(nc, ident16)

            # state Mt (= M^T) per batch lives in PSUM and accumulates the
            # per-chunk updates; an SBUF f16 copy is refreshed each chunk.
            mt = []
            mt_ps = []
            for b, pool in ((0, ps_mt0), (1, ps_mt1)):
                t0 = mtinitp.tile([128, 2, DV], F32, tag=f"mt0f{b}")
                nc.sync.dma_start(
                    out=t0, in_=memT[b].rearrange("(j p) v -> p j v", p=128)
                )
                ps = pool.tile([128, 2, DV], F32, tag=f"mtps{b}")
                nc.tensor.matmul(ps.rearrange("p j v -> p (j v)"), ident32,
                                 t0.rearrange("p j v -> p (j v)"),
                                 start=True, stop=False,
                                 skip_group_check=True)
                t = mtp.tile([128, 2, DV], F16, tag=f"mt{b}")
                if b == 0:
                    nc.vector.tensor_copy(t, ps)
                else:
                    nc.scalar.copy(t, ps)
                mt.append(t)
                mt_ps.append(ps)

            arts = {}

            def st_load(bi):
                kn4 = kv.tile([128, 2, 4, DK], F16, tag="kn4")
                knt4 = ktp.tile([128, 2, 2, 512], F16, tag="knt4")
                v4 = vv.tile([128, 2, 4, DV], F16, tag="v4")
                for b in range(BLOC):
                    nc.sync.dma_start(
                        out=kn4[:, b],
                        in_=key_d[b, bi * 512:(bi + 1) * 512, :].rearrange(
                            "(i p) k -> p i k", p=128),
                    )
                    nc.sync.dma_start(
                        out=knt4[:, b],
                        in_=keyT_d[b, :, bi * 512:(bi + 1) * 512].rearrange(
                            "(j p) s -> p j s", p=128),
                    )
                    nc.sync.dma_start(
                        out=v4[:, b],
                        in_=val_d[b, bi * 512:(bi + 1) * 512, :].rearrange(
                            "(i p) v -> p i v", p=128),
                    )
                for i in range(4):
                    c = bi * 4 + i
                    arts[c] = dict(
                        c=c,
                        Kn=kn4[:, :, i, :],                 # [128, 2, DK]
                        KnT=knt4[:, :, :, ts(i, 128)],      # [128, 2, 2, 128]
                        Vt=v4[:, :, i, :],                  # [128, 2, DV]
                    )

            def st_gram(a):
                a_ps = ps_gpw.tile([128, 2, 256], F32, tag="gpw")
                for b in range(BLOC):
                    for j in range(2):
                        nc.tensor.matmul(
                            a_ps[:, b, 0:128], a["KnT"][:, b, j, :],
                            a["KnT"][:, b, j, :],
                            start=(j == 0), stop=(j == 1),
                        )
                an = anp.tile([128, 2, 128], F16, tag="an")
                nc.scalar.mul(an, a_ps[:, :, 0:128], -AC)
                ln = mskp.tile([128, 2, 128], F16, tag="ln")
                nc.gpsimd.affine_select(
                    out=ln, in_=an, compare_op=AOP.is_gt, fill=0.0,
                    base=0, pattern=[[0, 2], [-1, 128]], channel_multiplier=1,
                )
                ltn = mskp.tile([128, 2, 128], F16, tag="ltn")
                nc.gpsimd.affine_select(
                    out=ltn, in_=an, compare_op=AOP.is_gt, fill=0.0,
                    base=0, pattern=[[0, 2], [1, 128]], channel_multiplier=-1,
                )
                a["ln"], a["ltn"] = ln, ltn

            def st_p2(a):
                ps = ps_gpw.tile([128, 2, 256], F32, tag="gpw")
                for b in range(BLOC):
                    nc.tensor.matmul(ps[:, b, 0:128],
                                     a["ltn"][:, b, :], a["ln"][:, b, :])
                    nc.tensor.matmul(ps[:, b, 128:256],
                                     a["ln"][:, b, :], a["ltn"][:, b, :])
                p2 = p2p.tile([128, 2, 256], F16, tag="p2")
                nc.vector.tensor_copy(p2, ps)
                a["l2"] = p2[:, :, 0:128]
                a["lt2"] = p2[:, :, 128:256]

            def st_p4(a):
                ps = ps_mid.tile([128, 2, 256], F32, tag="mid")
                for b in range(BLOC):
                    nc.tensor.matmul(ps[:, b, 0:128],
                                     a["lt2"][:, b, :], a["l2"][:, b, :])
                    nc.tensor.matmul(ps[:, b, 128:256],
                                     a["l2"][:, b, :], a["lt2"][:, b, :])
                p4 = p4p.tile([128, 2, 256], F16, tag="p4")
                nc.scalar.copy(p4, ps)
                a["l4"] = p4[:, :, 0:128]
                a["lt4"] = p4[:, :, 128:256]

            def st_l8q(a):
                ps = ps_mid.tile([128, 2, 256], F32, tag="mid")
                for b in range(BLOC):
                    nc.tensor.matmul(ps[:, b, 0:128],
                                     a["lt4"][:, b, :], a["l4"][:, b, :])
                q = qp.tile([128, 2, 128], F16, tag="q")
                nc.vector.scalar_tensor_tensor(
                    out=q, in0=ps[:, :, 0:128], scalar=1.0, in1=a["l4"],
                    op0=AOP.mult, op1=AOP.add,
                )
                nc.gpsimd.affine_select(
                    out=q, in_=q, compare_op=AOP.not_equal, fill=one_reg,
                    base=0, pattern=[[0, 2], [-1, 128]], channel_multiplier=1,
                )
                a["q"] = q

            def st_x2(a):
                # X2 = (I + L^2) Q  (ident-mm carries Q's unit diagonal)
                ps = ps_mid.tile([128, 2, 128], F32, tag="mid")
                for b in range(BLOC):
                    nc.tensor.matmul(ps[:, b, :], a["lt2"][:, b, :],
                                     a["q"][:, b, :], start=True, stop=False)
                    nc.tensor.matmul(ps[:, b, :], ident16,
                                     a["q"][:, b, :], start=False, stop=True)
                x2 = x2p.tile([128, 2, 128], F16, tag="x2")
                nc.vector.tensor_copy(x2, ps)
                a["x2"] = x2

            def st_t(a):
                # T = (I - L) X2  (ident-mm carries X2's unit diagonal)
                ps = ps_mid.tile([128, 2, 128], F32, tag="mid")
                for b in range(BLOC):
                    nc.tensor.matmul(ps[:, b, :], a["ltn"][:, b, :],
                                     a["x2"][:, b, :], start=True, stop=False)
                    nc.tensor.matmul(ps[:, b, :], ident16,
                                     a["x2"][:, b, :], start=False, stop=True)
                t = ttp.tile([128, 2, 128], F16, tag="t")
                nc.scalar.copy(t, ps)
                a["t"] = t

            def st_w(a):
                ps = ps_gpw.tile([128, 2, 256], F32, tag="gpw")
                for b in range(BLOC):
                    nc.tensor.matmul(ps[:, b, :], a["t"][:, b, :],
                                     a["Kn"][:, b, :])
                w = wp.tile([128, 2, 256], F16, tag="w")
                nc.scalar.mul(w, ps, LR)
                a["w"] = w

            def st_y(a, b):
                y = ps_y.tile([128, DV], F32, tag="y")
                for j in range(2):
                    nc.tensor.matmul(
                        y, a["KnT"][:, b, j, :], mt[b][:, j, :],
                        start=(j == 0), stop=(j == 1),
                    )
                a[f"y{b}"] = y

            def st_r(a, b):
                r = statep.tile([128, DV], F16, tag=f"r{b}")
                nc.vector.scalar_tensor_tensor(
                    out=r, in0=a[f"y{b}"], scalar=-10.0 * AC,
                    in1=a["Vt"][:, b, :], op0=AOP.mult, op1=AOP.add,
                )
                a[f"r{b}"] = r

            def st_mtupd(a, b):
                last = a["c"] == nch - 1
                for j in range(2):
                    nc.tensor.matmul(
                        mt_ps[b][:, j, :], a["w"][:, b, ts(j, 128)],
                        a[f"r{b}"],
                        start=False, stop=last, skip_group_check=True,
                    )

            def st_mtcopy(a, b):
                mt_new = mtp.tile([128, 2, DV], F16, tag=f"mt{b}")
                if b == 0:
                    nc.vector.tensor_copy(mt_new, mt_ps[b])
                else:
                    nc.scalar.copy(mt_new, mt_ps[b])
                mt[b] = mt_new

            # modulo software pipeline over chunks; stage offsets chosen so
            # each iteration's PE queue interleaves independent precompute
            # with this chunk's sequential state ops.
            for it in range(-6, nch + 1):
                if (it + 6) % 4 == 0 and (it + 6) // 4 < nbk:
                    st_load((it + 6) // 4)
                if 0 <= it < nch:
                    st_y(arts[it], 0)
                    st_r(arts[it], 0)
                if 0 <= it - 1 < nch:
                    st_y(arts[it - 1], 1)
                    st_r(arts[it - 1], 1)
                if 0 <= it + 5 < nch:
                    st_gram(arts[it + 5])
                if 0 <= it + 4 < nch:
                    st_p2(arts[it + 4])
                if 0 <= it < nch:
                    st_mtupd(arts[it], 0)
                    st_mtcopy(arts[it], 0)
                if 0 <= it - 1 < nch:
                    st_mtupd(arts[it - 1], 1)
                    st_mtcopy(arts[it - 1], 1)
                if 0 <= it + 3 < nch:
                    st_p4(arts[it + 3])
                if 0 <= it + 2 < nch:
                    st_l8q(arts[it + 2])
                if 0 <= it + 1 < nch:
                    st_x2(arts[it + 1])
                    st_t(arts[it + 1])
                    st_w(arts[it + 1])
                if 0 <= it - 2:
                    arts.pop(it - 2, None)

            for b in range(BLOC):
                fin = mtinitp.tile([128, 2, DV], F32, tag=f"fin{b}")
                nc.vector.tensor_copy(fin, mt_ps[b])
                nc.sync.dma_start(
                    out=outT[b].rearrange("(j p) v -> p j v", p=128),
                    in_=fin,
                )
    if split:
        _split_waits(nc)
    return nc


_NC_CACHE = {}

# test-harness hooks (the grading harness just calls kernel())
TRACE = False
LAST_RESULT = None


def _get_nc(s_loc=S):
    if s_loc not in _NC_CACHE:
        _NC_CACHE[s_loc] = build_nc(s_loc)
    return _NC_CACHE[s_loc]


def kernel(memory, key, value):
    global LAST_RESULT
    memory = np.ascontiguousarray(np.asarray(memory), dtype=np.float32)
    key = np.asarray(key, dtype=np.float32)
    # normalize keys on host (k / (||k|| + eps)); the recurrence only ever
    # uses normalized keys, so this is input layout prep for the kernel
    nrm = np.sqrt(np.einsum("bsk,bsk->bs", key, key))[..., None]
    kn = (key / (nrm + 1e-6)).astype(np.float16)
    knT = np.ascontiguousarray(kn.transpose(0, 2, 1))
    kn = np.ascontiguousarray(kn)
    value = np.ascontiguousarray(np.asarray(value), dtype=np.float16)
    s_loc = key.shape[1]
    nc = _get_nc(s_loc)
    memT = np.ascontiguousarray(memory.transpose(0, 2, 1))
    in_maps = []
    for i in range(NCORES):
        sl = slice(i * BLOC, (i + 1) * BLOC)
        in_maps.append({
            "memT": memT[sl],
            "key": np.ascontiguousarray(kn[sl]),
            "keyT": np.ascontiguousarray(knT[sl]),
            "value": np.ascontiguousarray(value[sl]),
        })
    res = run_bass_kernel_spmd(nc, in_maps, list(range(NCORES)), trace=TRACE)
    LAST_RESULT = res
    outs = [res.results[i]["outT"] for i in range(NCORES)]
    out = np.concatenate(outs, axis=0)          # (16, DK, DV) = M^T
    return np.ascontiguousarray(out.transpose(0, 2, 1))
